# revision 24
# baseline (speedup 1.0000x reference)
"""Trainium2 Bass kernel for nn_AttModel (masked attention GNN message passing).

Contract: kernel(**inputs) takes the FULL unsharded inputs (x [8,2048,128],
mask [8,2048,2048], Wv/Wk/Wq [128,128], bv/bk/bq [128]) and returns the full
output [8, 2048, 128] float32.

Strategy: data-parallel over batch B=8 across the 8 NeuronCores; weights
replicated. Per core, fully transposed dataflow (scores computed as S^T):

  qT/kT/vT = relu(W xT + b) as [h, n] bf16  (bf16 projections, f32 PSUM)
  v_all    = vT block-transposed to [n, h] via PE transposes
  per i-chunk (1024) x j-stripe-group (4 stripes of 128):
    sT   = kT_j^T @ qT_chunk          (PE, bf16, PSUM f32)
    e    = exp(sT)                    (ACT, PSUM -> SBUF bf16)
    p    = e "masked to ~0"           (applied DURING the mask DMA via
                                       SWDGE accum_op=add on the int16
                                       BIT PATTERN of e -- mask never
                                       touches a compute engine)
    outT += v_j^T @ p                 (PE, PSUM accumulation over stripes)
    tree: w_g = (p0+p1)+(p2+p3)       (DVE pair-sum)
    rowsum += 1^T @ w_g               (PE, M=1 matmul on tree output)
  Host: out_b = (outT / rowsum)^T

The group back-end (out matmuls + tree + rowsum) is software-pipelined one
group behind the front-end (scores + exp + mask DMA) so the PE never
head-of-line blocks on the mask DMA completion and stays HAM-warm.

Masking trick: q,k >= 0 post-relu so s >= 0 and e = exp(s) >= 1.0, i.e. the
bf16 bits of e are >= 0x3F80. The mask ships as int16 {0, -0x3F80}; the SWDGE
DMA integer-adds it onto the bitcast e tile, so masked entries become
bits-0x3F80 = e * 2^-127 (~1e-30, vanishes in f32 accumulation against
unmasked terms >= 1) while unmasked entries are untouched (+0 exact).
"""

from contextlib import ExitStack

import numpy as np
import ml_dtypes

import concourse.bass as bass
import concourse.bacc as bacc
import concourse.tile as tile
from concourse import mybir
from concourse import bass_utils

B = 8
P = 128
N = 2048
HID = 128
DIN = 128
ICH = 1024       # i-chunk width
NCH = N // ICH   # 2 i-chunks
NG = 2           # stripe groups per chunk
GS = 8           # stripes per group

f32 = mybir.dt.float32
bf = mybir.dt.bfloat16
i16 = mybir.dt.int16
AF = mybir.ActivationFunctionType
ALU = mybir.AluOpType

MASK_SUB = -0x3F80  # subtract from bf16 bits of e where mask==0

_NC_CACHE = {}


def _attention_tile_kernel(ctx, tc, outT, wsum, xn, maskh, cf32, cbf):
    nc = tc.nc

    consts = ctx.enter_context(tc.tile_pool(name="consts", bufs=1))
    big = ctx.enter_context(tc.tile_pool(name="big", bufs=1))

    # x arrives pre-transposed [d, n] from the host; plain DMAs (the
    # DMA-transpose path serializes against all other DMAs). First half
    # gates the first projections, so it is issued first and split.
    xT_sb = big.tile([P, N], bf)
    nc.sync.dma_start(out=xT_sb[:, 0:ICH], in_=xn[:, 0:ICH])
    blob_b = consts.tile([P, 512], bf)
    nc.scalar.dma_start(out=blob_b, in_=cbf)
    blob_f = consts.tile([P, 4], f32)
    nc.scalar.dma_start(out=blob_f, in_=cf32)
    nc.sync.dma_start(out=xT_sb[:, ICH:N], in_=xn[:, ICH:N])

    biases = {"q": blob_f[:, 0:1], "k": blob_f[:, 1:2], "v": blob_f[:, 2:3]}
    wTs = {"q": blob_b[:, 0:P], "k": blob_b[:, P:2 * P], "v": blob_b[:, 2 * P:3 * P]}
    idb = blob_b[:, 3 * P:4 * P]

    mask_pool = ctx.enter_context(tc.tile_pool(name="maskp", bufs=1))
    # Prefetch the per-group q3 mask slices on the plain (fast) DMA path;
    # they have no dependencies so they load during the projection preamble.
    mask_dve = {}
    for cc in range(NCH):
        for gg in range(NG):
            qs = range(GS) if (cc, gg) == (NCH - 1, NG - 1) else (5, 6, 7)
            for qq in qs:
                mt = mask_pool.tile([P, ICH], i16, tag=f"m{cc}{gg}{qq}",
                                    name=f"mask_{cc}_{gg}_{qq}")
                j0 = (gg * GS + qq) * P
                nc.sync.dma_start(
                    out=mt, in_=maskh[j0:j0 + P, cc * ICH:(cc + 1) * ICH])
                mask_dve[(cc, gg, qq)] = mt

    qT = big.tile([P, N], bf)
    kT = big.tile([P, N], bf)
    vT = big.tile([P, N], bf)
    v_all = big.tile([P, N], bf)    # [n, h] in 128-blocks: v_all[:, jt*128+h]

    s_psum = ctx.enter_context(tc.tile_pool(name="spsum", bufs=2, space="PSUM"))
    o_psum = ctx.enter_context(tc.tile_pool(name="opsum", bufs=1, space="PSUM"))
    p_psum = ctx.enter_context(tc.tile_pool(name="ppsum", bufs=1, space="PSUM"))
    e_pool = ctx.enter_context(tc.tile_pool(name="ep", bufs=3))
    # ea: [P,5*ICH] CCE-masked; ec: [P,3*ICH] DVE-masked (separate tiles)
    tree_pool = ctx.enter_context(tc.tile_pool(name="treep", bufs=2))
    out_sb_pool = ctx.enter_context(tc.tile_pool(name="outsbp", bufs=2))

    def proj(nm, c, dest, act_engine):
        """dest[:, c*1024:(c+1)*1024] = relu(W^T x + b) for n-chunk c."""
        ps = p_psum.tile([P, ICH], f32, tag="p", name=f"proj_{nm}{c}")
        for h in range(2):
            nc.tensor.matmul(ps[:, h * 512:(h + 1) * 512], lhsT=wTs[nm],
                             rhs=xT_sb[:, c * ICH + h * 512:c * ICH + (h + 1) * 512],
                             start=True, stop=True)
        dslice = dest[:, c * ICH:(c + 1) * ICH]
        if act_engine:
            nc.scalar.activation(out=dslice, in_=ps, func=AF.Relu,
                                 bias=biases[nm], scale=1.0)
        else:
            nc.vector.tensor_scalar(out=dslice, in0=ps, scalar1=biases[nm],
                                    scalar2=0.0, op0=ALU.add, op1=ALU.max)

    def transp_half(half):
        """v_all[:, half*1024:(half+1)*1024] = blockwise vT^T (8 blocks)."""
        tp = p_psum.tile([P, ICH], bf, tag="p", name=f"vtp{half}")
        for b8 in range(8):
            jt = half * 8 + b8
            nc.tensor.transpose(tp[:, b8 * P:(b8 + 1) * P],
                                vT[:, jt * P:(jt + 1) * P], idb)
        nc.vector.tensor_copy(out=v_all[:, half * ICH:(half + 1) * ICH], in_=tp)

    o_tiles = {}
    e_tiles = {}

    def cce_mask(c, g, q0, nstripes, e_a):
        """int16-add the mask onto stripes [q0, q0+nstripes) during a DMA.

        The accum source runs must be strided 1024-elem rows (adjacent runs
        aggregate past the CCE element limit and corrupt data)."""
        j0 = (g * GS + q0) * P
        m_ap = maskh[j0:j0 + nstripes * P,
                     c * ICH:(c + 1) * ICH].rearrange("(s p) i -> p s i", p=P)
        nc.gpsimd.dma_start(
            out=e_a[:, q0 * ICH:(q0 + nstripes) * ICH].bitcast(i16),
            in_=m_ap, accum_op=ALU.add)

    def front(c, g):
        """Scores + exp for 8 stripes. Stripes 0-4 land in e_a and are
        masked by two CCE DMAs issued after exp 4 (exps 5-7 write the
        separate e_c tile, so the whole-tile DMA dependency never stalls
        the exp stream); stripes 5-7 are DVE-masked. The final group is
        all-DVE so the tail never waits on the accumulate DMA."""
        i0 = c * ICH
        last = (c, g) == (NCH - 1, NG - 1)
        e_a = e_pool.tile([P, 5 * ICH], bf, tag="ea", name=f"ea_{c}_{g}")
        e_cs = [e_pool.tile([P, ICH], bf, tag=f"ec{j}", name=f"ec{j}_{c}_{g}")
                for j in range(3)]
        e_tiles[(c, g)] = (e_a, e_cs)
        for q in range(GS):
            jt = g * GS + q
            s_ps = s_psum.tile([P, ICH], f32, tag="s", name=f"s_{c}_{jt}")
            for h in range(2):
                nc.tensor.matmul(
                    s_ps[:, h * 512:(h + 1) * 512],
                    lhsT=kT[:, jt * P:(jt + 1) * P],
                    rhs=qT[:, i0 + h * 512:i0 + (h + 1) * 512],
                    start=True, stop=True)
            dst = (e_a[:, q * ICH:(q + 1) * ICH] if q < 5 else e_cs[q - 5])
            nc.scalar.activation(out=dst, in_=s_ps, func=AF.Exp)
            if q >= 5:
                # per-stripe tiles: the in-place add never blocks later exps
                eq = dst.bitcast(i16)
                nc.vector.tensor_tensor(out=eq, in0=eq,
                                        in1=mask_dve.pop((c, g, q)),
                                        op=ALU.add)
            if q == 4:
                if last:
                    # batch the e_a adds here; they run on the DVE while
                    # the (independent) e_c exps stream on
                    for qq in range(5):
                        eq = e_a[:, qq * ICH:(qq + 1) * ICH].bitcast(i16)
                        nc.vector.tensor_tensor(
                            out=eq, in0=eq, in1=mask_dve.pop((c, g, qq)),
                            op=ALU.add)
                else:
                    cce_mask(c, g, 0, 3, e_a)
                    cce_mask(c, g, 3, 2, e_a)

    def back(c, g):
        """Out-matmul accumulation + rowsum tree for a completed group."""
        e_a, e_cs = e_tiles.pop((c, g))
        if g == 0:
            o_tiles[c] = o_psum.tile([P, ICH], f32, tag="o", name=f"o_{c}")
        o_ps = o_tiles[c]
        for q in range(GS):
            jt = g * GS + q
            if q < 5:
                rhs_t = lambda h: e_a[:, q * ICH + h * 512:
                                      q * ICH + (h + 1) * 512]
            else:
                rhs_t = lambda h: e_cs[q - 5][:, h * 512:(h + 1) * 512]
            for h in range(2):
                nc.tensor.matmul(
                    o_ps[:, h * 512:(h + 1) * 512],
                    lhsT=v_all[:, jt * P:(jt + 1) * P],
                    rhs=rhs_t(h),
                    start=(jt == 0), stop=(jt == 15))
        # one pair-sum level on the DVE (4 blocks); the rest of the rowsum
        # reduction (blocks x partitions) happens on the host
        t_g = tree_pool.tile([P, 4 * ICH], bf, tag="t", name=f"t_{c}_{g}")
        nc.vector.tensor_tensor(out=t_g[:, 0:ICH], in0=e_a[:, 0:ICH],
                                in1=e_a[:, ICH:2 * ICH], op=ALU.add)
        nc.vector.tensor_tensor(out=t_g[:, ICH:2 * ICH],
                                in0=e_a[:, 2 * ICH:3 * ICH],
                                in1=e_a[:, 3 * ICH:4 * ICH], op=ALU.add)
        nc.vector.tensor_tensor(out=t_g[:, 2 * ICH:3 * ICH],
                                in0=e_cs[0], in1=e_cs[1], op=ALU.add)
        nc.vector.tensor_tensor(out=t_g[:, 3 * ICH:4 * ICH],
                                in0=e_a[:, 4 * ICH:5 * ICH],
                                in1=e_cs[2], op=ALU.add)
        widx = (c * NG + g) * P
        nc.sync.dma_start(out=wsum[widx:widx + P, :], in_=t_g)

    def flush(c):
        i0 = c * ICH
        o_ps = o_tiles.pop(c)
        out_sb = out_sb_pool.tile([P, ICH], bf, tag="osb", name=f"osb_{c}")
        if c == NCH - 1:
            # ACT is idle once the final exp retires
            nc.scalar.copy(out=out_sb, in_=o_ps)
        else:
            nc.vector.tensor_copy(out=out_sb, in_=o_ps)
        nc.sync.dma_start(out=outT[:, i0:i0 + ICH], in_=out_sb)

    # Critical path into the main loop: q/k chunk-0 projections (ACT relu).
    proj("q", 0, qT, True)
    proj("k", 0, kT, True)

    # Off-critical-path setup, interleaved into the early groups (the PE has
    # spare cycles there since the back-end lags by one group).
    extras = {
        (0, 0): [lambda: proj("k", 1, kT, False),
                 lambda: proj("q", 1, qT, False),
                 lambda: proj("v", 0, vT, False), lambda: transp_half(0)],
        (0, 1): [lambda: proj("v", 1, vT, False), lambda: transp_half(1)],
    }

    # Explicit scheduling windows: tile_set_cur_wait pins each pipeline
    # stage into its own window in the scheduler's simulated timeline, so
    # group g's scores/exp always precede the (lagged) back-end of group
    # g-LAG in every engine queue -- the scheduler's DMA cost model
    # underestimates the accumulate path and would otherwise reorder.
    LAG = 1
    seq = [(c, g) for c in range(NCH) for g in range(NG)]
    for idx, cg in enumerate(seq):
        tc.tile_set_cur_wait(1.0 * (1 + idx))
        front(*cg)
        if cg in extras:
            # setup work rides in a sub-window so the scheduler orders it
            # after this window's scores and the previous group's back-end
            tc.tile_set_cur_wait(1.0 * (1 + idx) + 0.5)
            for fn in extras[cg]:
                fn()
            tc.tile_set_cur_wait(1.0 * (1 + idx))
        if idx >= LAG:
            done = seq[idx - LAG]
            back(*done)
            if done[1] == NG - 1:
                flush(done[0])
    for j, done in enumerate(seq[-LAG:]):
        tc.tile_set_cur_wait(1.0 * (1 + len(seq) + j))
        back(*done)
        if done[1] == NG - 1:
            flush(done[0])


def _build_nc():
    if "nc" in _NC_CACHE:
        return _NC_CACHE["nc"]
    nc = bacc.Bacc("TRN2", target_bir_lowering=False, debug=False, num_devices=B)
    xn = nc.dram_tensor("xn", [DIN, N], bf, kind="ExternalInput").ap()
    maskh = nc.dram_tensor("maskh", [N, N], i16, kind="ExternalInput").ap()
    cf32 = nc.dram_tensor("cf32", [P, 4], f32, kind="ExternalInput").ap()
    cbf = nc.dram_tensor("cbf", [P, 512], bf, kind="ExternalInput").ap()
    outT = nc.dram_tensor("outT", [HID, N], bf, kind="ExternalOutput").ap()
    wsum = nc.dram_tensor("wsum", [NCH * NG * P, (GS // 2) * ICH], bf,
                          kind="ExternalOutput").ap()

    with tile.TileContext(nc) as tc:
        with ExitStack() as ctx:
            _attention_tile_kernel(ctx, tc, outT, wsum, xn, maskh, cf32, cbf)
    nc.compile()
    _NC_CACHE["nc"] = nc
    return nc


def build_nc():
    return _build_nc()


def make_in_maps(x, mask, Wv, bv, Wk, bk, Wq, bq):
    x = np.asarray(x, dtype=np.float32)
    mask = np.asarray(mask, dtype=np.float32)
    Wv = np.asarray(Wv, dtype=np.float32)
    bv = np.asarray(bv, dtype=np.float32)
    Wk = np.asarray(Wk, dtype=np.float32)
    bk = np.asarray(bk, dtype=np.float32)
    Wq = np.asarray(Wq, dtype=np.float32)
    bq = np.asarray(bq, dtype=np.float32)

    cf32 = np.zeros((P, 4), np.float32)
    cf32[:, 0], cf32[:, 1], cf32[:, 2] = bq, bk, bv
    cbf = np.concatenate(
        [Wq.T, Wk.T, Wv.T, np.eye(P, dtype=np.float32)],
        axis=1).astype(ml_dtypes.bfloat16)
    cbf = np.ascontiguousarray(cbf)

    in_maps = []
    for c in range(B):
        maskadd = np.where(mask[c].T >= 0.5, 0, MASK_SUB).astype(np.int16)
        in_maps.append({
            "xn": np.ascontiguousarray(x[c].T).astype(ml_dtypes.bfloat16),
            "maskh": np.ascontiguousarray(maskadd),
            "cf32": cf32, "cbf": cbf,
        })
    return in_maps


def kernel(x, mask, Wv, bv, Wk, bk, Wq, bq):
    nc = _build_nc()
    in_maps = make_in_maps(x, mask, Wv, bv, Wk, bk, Wq, bq)
    res = bass_utils.run_bass_kernel_spmd(nc, in_maps, core_ids=list(range(B)),
                                          trace=False)
    out = np.empty((B, N, HID), dtype=np.float32)
    for c in range(B):
        outT = res.results[c]["outT"].astype(np.float32)
        w = res.results[c]["wsum"].astype(np.float32)
        w = w.reshape(NCH, NG * P, GS // 2, ICH)
        rowsum = w.sum(axis=(1, 2)).reshape(1, N)
        rowsum = np.where(rowsum == 0.0, 1.0, rowsum)
        out[c] = (outT / rowsum).T
    return out


# revision 25
# speedup vs baseline: 1.0258x; 1.0258x over previous
"""Trainium2 Bass kernel for nn_AttModel (masked attention GNN message passing).

Contract: kernel(**inputs) takes the FULL unsharded inputs (x [8,2048,128],
mask [8,2048,2048], Wv/Wk/Wq [128,128], bv/bk/bq [128]) and returns the full
output [8, 2048, 128] float32.

Strategy: data-parallel over batch B=8 across the 8 NeuronCores; weights
replicated. Per core, fully transposed dataflow (scores computed as S^T):

  qT/kT/vT = relu(W xT + b) as [h, n] bf16  (bf16 projections, f32 PSUM)
  v_all    = vT block-transposed to [n, h] via PE transposes
  per i-chunk (1024) x j-stripe-group (4 stripes of 128):
    sT   = kT_j^T @ qT_chunk          (PE, bf16, PSUM f32)
    e    = exp(sT)                    (ACT, PSUM -> SBUF bf16)
    p    = e "masked to ~0"           (applied DURING the mask DMA via
                                       SWDGE accum_op=add on the int16
                                       BIT PATTERN of e -- mask never
                                       touches a compute engine)
    outT += v_j^T @ p                 (PE, PSUM accumulation over stripes)
    tree: w_g = (p0+p1)+(p2+p3)       (DVE pair-sum)
    rowsum += 1^T @ w_g               (PE, M=1 matmul on tree output)
  Host: out_b = (outT / rowsum)^T

The group back-end (out matmuls + tree + rowsum) is software-pipelined one
group behind the front-end (scores + exp + mask DMA) so the PE never
head-of-line blocks on the mask DMA completion and stays HAM-warm.

Masking trick: q,k >= 0 post-relu so s >= 0 and e = exp(s) >= 1.0, i.e. the
bf16 bits of e are >= 0x3F80. The mask ships as int16 {0, -0x3F80}; the SWDGE
DMA integer-adds it onto the bitcast e tile, so masked entries become
bits-0x3F80 = e * 2^-127 (~1e-30, vanishes in f32 accumulation against
unmasked terms >= 1) while unmasked entries are untouched (+0 exact).
"""

from contextlib import ExitStack

import numpy as np
import ml_dtypes

import concourse.bass as bass
import concourse.bacc as bacc
import concourse.tile as tile
from concourse import mybir
from concourse import bass_utils

B = 8
P = 128
N = 2048
HID = 128
DIN = 128
ICH = 1024       # i-chunk width
NCH = N // ICH   # 2 i-chunks
NG = 2           # stripe groups per chunk
GS = 8           # stripes per group

f32 = mybir.dt.float32
bf = mybir.dt.bfloat16
i16 = mybir.dt.int16
AF = mybir.ActivationFunctionType
ALU = mybir.AluOpType

MASK_SUB = -0x3F80  # subtract from bf16 bits of e where mask==0

_NC_CACHE = {}


def _attention_tile_kernel(ctx, tc, outT, wsum, xn, maskh, cf32, cbf):
    nc = tc.nc

    consts = ctx.enter_context(tc.tile_pool(name="consts", bufs=1))
    big = ctx.enter_context(tc.tile_pool(name="big", bufs=1))

    # x arrives pre-transposed [d, n] from the host; plain DMAs (the
    # DMA-transpose path serializes against all other DMAs). First half
    # gates the first projections, so it is issued first and split.
    xT_sb = big.tile([P, N], bf)
    nc.sync.dma_start(out=xT_sb[:, 0:ICH], in_=xn[:, 0:ICH])
    blob_b = consts.tile([P, 512], bf)
    nc.scalar.dma_start(out=blob_b, in_=cbf)
    blob_f = consts.tile([P, 4], f32)
    nc.scalar.dma_start(out=blob_f, in_=cf32)
    nc.sync.dma_start(out=xT_sb[:, ICH:N], in_=xn[:, ICH:N])

    biases = {"q": blob_f[:, 0:1], "k": blob_f[:, 1:2], "v": blob_f[:, 2:3]}
    wTs = {"q": blob_b[:, 0:P], "k": blob_b[:, P:2 * P], "v": blob_b[:, 2 * P:3 * P]}
    idb = blob_b[:, 3 * P:4 * P]

    mask_pool = ctx.enter_context(tc.tile_pool(name="maskp", bufs=1))
    # Prefetch the per-group q3 mask slices on the plain (fast) DMA path;
    # they have no dependencies so they load during the projection preamble.
    mask_dve = {}
    for cc in range(NCH):
        for gg in range(NG):
            qs = range(GS) if (cc, gg) == (NCH - 1, NG - 1) else (5, 6, 7)
            for qq in qs:
                mt = mask_pool.tile([P, ICH], i16, tag=f"m{cc}{gg}{qq}",
                                    name=f"mask_{cc}_{gg}_{qq}")
                j0 = (gg * GS + qq) * P
                nc.sync.dma_start(
                    out=mt, in_=maskh[j0:j0 + P, cc * ICH:(cc + 1) * ICH])
                mask_dve[(cc, gg, qq)] = mt

    qT = big.tile([P, N], bf)
    kT = big.tile([P, N], bf)
    vT = big.tile([P, N], bf)
    v_all = big.tile([P, N], bf)    # [n, h] in 128-blocks: v_all[:, jt*128+h]

    s_psum = ctx.enter_context(tc.tile_pool(name="spsum", bufs=2, space="PSUM"))
    o_psum = ctx.enter_context(tc.tile_pool(name="opsum", bufs=1, space="PSUM"))
    p_psum = ctx.enter_context(tc.tile_pool(name="ppsum", bufs=1, space="PSUM"))
    e_pool = ctx.enter_context(tc.tile_pool(name="ep", bufs=3))
    # ea: [P,5*ICH] CCE-masked; ec: [P,3*ICH] DVE-masked (separate tiles)
    tree_pool = ctx.enter_context(tc.tile_pool(name="treep", bufs=2))
    out_sb_pool = ctx.enter_context(tc.tile_pool(name="outsbp", bufs=2))

    def proj(nm, c, dest, act_engine):
        """dest[:, c*1024:(c+1)*1024] = relu(W^T x + b) for n-chunk c."""
        ps = p_psum.tile([P, ICH], f32, tag="p", name=f"proj_{nm}{c}")
        for h in range(2):
            nc.tensor.matmul(ps[:, h * 512:(h + 1) * 512], lhsT=wTs[nm],
                             rhs=xT_sb[:, c * ICH + h * 512:c * ICH + (h + 1) * 512],
                             start=True, stop=True)
        dslice = dest[:, c * ICH:(c + 1) * ICH]
        if act_engine:
            nc.scalar.activation(out=dslice, in_=ps, func=AF.Relu,
                                 bias=biases[nm], scale=1.0)
        else:
            nc.vector.tensor_scalar(out=dslice, in0=ps, scalar1=biases[nm],
                                    scalar2=0.0, op0=ALU.add, op1=ALU.max)

    def transp_half(half):
        """v_all[:, half*1024:(half+1)*1024] = blockwise vT^T (8 blocks)."""
        tp = p_psum.tile([P, ICH], bf, tag="p", name=f"vtp{half}")
        for b8 in range(8):
            jt = half * 8 + b8
            nc.tensor.transpose(tp[:, b8 * P:(b8 + 1) * P],
                                vT[:, jt * P:(jt + 1) * P], idb)
        nc.vector.tensor_copy(out=v_all[:, half * ICH:(half + 1) * ICH], in_=tp)

    o_tiles = {}
    e_tiles = {}

    def cce_mask(c, g, q0, nstripes, e_a):
        """int16-add the mask onto stripes [q0, q0+nstripes) during a DMA.

        The accum source runs must be strided 1024-elem rows (adjacent runs
        aggregate past the CCE element limit and corrupt data)."""
        j0 = (g * GS + q0) * P
        m_ap = maskh[j0:j0 + nstripes * P,
                     c * ICH:(c + 1) * ICH].rearrange("(s p) i -> p s i", p=P)
        nc.gpsimd.dma_start(
            out=e_a[:, q0 * ICH:(q0 + nstripes) * ICH].bitcast(i16),
            in_=m_ap, accum_op=ALU.add)

    def front(c, g):
        """Scores + exp for 8 stripes. Stripes 0-4 land in e_a and are
        masked by two CCE DMAs issued after exp 4 (exps 5-7 write the
        separate e_c tile, so the whole-tile DMA dependency never stalls
        the exp stream); stripes 5-7 are DVE-masked. The final group is
        all-DVE so the tail never waits on the accumulate DMA."""
        i0 = c * ICH
        last = (c, g) == (NCH - 1, NG - 1)
        e_a = e_pool.tile([P, 5 * ICH], bf, tag="ea", name=f"ea_{c}_{g}")
        e_cs = [e_pool.tile([P, ICH], bf, tag=f"ec{j}", name=f"ec{j}_{c}_{g}")
                for j in range(3)]
        e_tiles[(c, g)] = (e_a, e_cs)
        for q in range(GS):
            jt = g * GS + q
            s_ps = s_psum.tile([P, ICH], f32, tag="s", name=f"s_{c}_{jt}")
            for h in range(2):
                nc.tensor.matmul(
                    s_ps[:, h * 512:(h + 1) * 512],
                    lhsT=kT[:, jt * P:(jt + 1) * P],
                    rhs=qT[:, i0 + h * 512:i0 + (h + 1) * 512],
                    start=True, stop=True)
            dst = (e_a[:, q * ICH:(q + 1) * ICH] if q < 5 else e_cs[q - 5])
            nc.scalar.activation(out=dst, in_=s_ps, func=AF.Exp)
            if q >= 5:
                # per-stripe tiles: the in-place add never blocks later exps
                eq = dst.bitcast(i16)
                nc.vector.tensor_tensor(out=eq, in0=eq,
                                        in1=mask_dve.pop((c, g, q)),
                                        op=ALU.add)
            if q == 4:
                if last:
                    # batch the e_a adds here; they run on the DVE while
                    # the (independent) e_c exps stream on
                    for qq in range(5):
                        eq = e_a[:, qq * ICH:(qq + 1) * ICH].bitcast(i16)
                        nc.vector.tensor_tensor(
                            out=eq, in0=eq, in1=mask_dve.pop((c, g, qq)),
                            op=ALU.add)
                else:
                    cce_mask(c, g, 0, 3, e_a)
                    cce_mask(c, g, 3, 2, e_a)

    def back(c, g):
        """Out-matmul accumulation + rowsum tree for a completed group."""
        e_a, e_cs = e_tiles.pop((c, g))
        if g == 0:
            o_tiles[c] = o_psum.tile([P, ICH], f32, tag="o", name=f"o_{c}")
        o_ps = o_tiles[c]
        for q in range(GS):
            jt = g * GS + q
            if q < 5:
                rhs_t = lambda h: e_a[:, q * ICH + h * 512:
                                      q * ICH + (h + 1) * 512]
            else:
                rhs_t = lambda h: e_cs[q - 5][:, h * 512:(h + 1) * 512]
            for h in range(2):
                nc.tensor.matmul(
                    o_ps[:, h * 512:(h + 1) * 512],
                    lhsT=v_all[:, jt * P:(jt + 1) * P],
                    rhs=rhs_t(h),
                    start=(jt == 0), stop=(jt == 15))
        # one pair-sum level on the DVE (4 blocks); the rest of the rowsum
        # reduction (blocks x partitions) happens on the host
        t_g = tree_pool.tile([P, 4 * ICH], bf, tag="t", name=f"t_{c}_{g}")
        nc.vector.tensor_tensor(out=t_g[:, 0:ICH], in0=e_a[:, 0:ICH],
                                in1=e_a[:, ICH:2 * ICH], op=ALU.add)
        nc.vector.tensor_tensor(out=t_g[:, ICH:2 * ICH],
                                in0=e_a[:, 2 * ICH:3 * ICH],
                                in1=e_a[:, 3 * ICH:4 * ICH], op=ALU.add)
        nc.vector.tensor_tensor(out=t_g[:, 2 * ICH:3 * ICH],
                                in0=e_cs[0], in1=e_cs[1], op=ALU.add)
        nc.vector.tensor_tensor(out=t_g[:, 3 * ICH:4 * ICH],
                                in0=e_a[:, 4 * ICH:5 * ICH],
                                in1=e_cs[2], op=ALU.add)
        widx = (c * NG + g) * P
        nc.sync.dma_start(out=wsum[widx:widx + P, :], in_=t_g)

    def flush(c):
        i0 = c * ICH
        o_ps = o_tiles.pop(c)
        out_sb = out_sb_pool.tile([P, ICH], bf, tag="osb", name=f"osb_{c}")
        if c == NCH - 1:
            # ACT is idle once the final exp retires
            nc.scalar.copy(out=out_sb, in_=o_ps)
        else:
            nc.vector.tensor_copy(out=out_sb, in_=o_ps)
        nc.sync.dma_start(out=outT[:, i0:i0 + ICH], in_=out_sb)

    # Critical path into the main loop: q/k chunk-0 projections (ACT relu).
    proj("q", 0, qT, True)
    proj("k", 0, kT, True)

    # Off-critical-path setup, interleaved into the early groups (the PE has
    # spare cycles there since the back-end lags by one group).
    extras = {
        (0, 0): [lambda: proj("k", 1, kT, False),
                 lambda: proj("q", 1, qT, False),
                 lambda: proj("v", 0, vT, False), lambda: transp_half(0)],
        (0, 1): [lambda: proj("v", 1, vT, False), lambda: transp_half(1)],
    }

    # Explicit scheduling windows: tile_set_cur_wait pins each pipeline
    # stage into its own window in the scheduler's simulated timeline, so
    # group g's scores/exp always precede the (lagged) back-end of group
    # g-LAG in every engine queue -- the scheduler's DMA cost model
    # underestimates the accumulate path and would otherwise reorder.
    LAG = 1
    seq = [(c, g) for c in range(NCH) for g in range(NG)]
    for idx, cg in enumerate(seq):
        tc.tile_set_cur_wait(1.0 * (1 + idx))
        front(*cg)
        for fn in extras.get(cg, ()):
            fn()
        if idx >= LAG:
            done = seq[idx - LAG]
            back(*done)
            if done[1] == NG - 1:
                flush(done[0])
    for j, done in enumerate(seq[-LAG:]):
        tc.tile_set_cur_wait(1.0 * (1 + len(seq) + j))
        back(*done)
        if done[1] == NG - 1:
            flush(done[0])


def _build_nc():
    if "nc" in _NC_CACHE:
        return _NC_CACHE["nc"]
    nc = bacc.Bacc("TRN2", target_bir_lowering=False, debug=False, num_devices=B)
    xn = nc.dram_tensor("xn", [DIN, N], bf, kind="ExternalInput").ap()
    maskh = nc.dram_tensor("maskh", [N, N], i16, kind="ExternalInput").ap()
    cf32 = nc.dram_tensor("cf32", [P, 4], f32, kind="ExternalInput").ap()
    cbf = nc.dram_tensor("cbf", [P, 512], bf, kind="ExternalInput").ap()
    outT = nc.dram_tensor("outT", [HID, N], bf, kind="ExternalOutput").ap()
    wsum = nc.dram_tensor("wsum", [NCH * NG * P, (GS // 2) * ICH], bf,
                          kind="ExternalOutput").ap()

    with tile.TileContext(nc) as tc:
        with ExitStack() as ctx:
            _attention_tile_kernel(ctx, tc, outT, wsum, xn, maskh, cf32, cbf)
    nc.compile()
    _NC_CACHE["nc"] = nc
    return nc


def build_nc():
    return _build_nc()


def make_in_maps(x, mask, Wv, bv, Wk, bk, Wq, bq):
    x = np.asarray(x, dtype=np.float32)
    mask = np.asarray(mask, dtype=np.float32)
    Wv = np.asarray(Wv, dtype=np.float32)
    bv = np.asarray(bv, dtype=np.float32)
    Wk = np.asarray(Wk, dtype=np.float32)
    bk = np.asarray(bk, dtype=np.float32)
    Wq = np.asarray(Wq, dtype=np.float32)
    bq = np.asarray(bq, dtype=np.float32)

    cf32 = np.zeros((P, 4), np.float32)
    cf32[:, 0], cf32[:, 1], cf32[:, 2] = bq, bk, bv
    cbf = np.concatenate(
        [Wq.T, Wk.T, Wv.T, np.eye(P, dtype=np.float32)],
        axis=1).astype(ml_dtypes.bfloat16)
    cbf = np.ascontiguousarray(cbf)

    in_maps = []
    for c in range(B):
        maskadd = np.where(mask[c].T >= 0.5, 0, MASK_SUB).astype(np.int16)
        in_maps.append({
            "xn": np.ascontiguousarray(x[c].T).astype(ml_dtypes.bfloat16),
            "maskh": np.ascontiguousarray(maskadd),
            "cf32": cf32, "cbf": cbf,
        })
    return in_maps


def kernel(x, mask, Wv, bv, Wk, bk, Wq, bq):
    nc = _build_nc()
    in_maps = make_in_maps(x, mask, Wv, bv, Wk, bk, Wq, bq)
    res = bass_utils.run_bass_kernel_spmd(nc, in_maps, core_ids=list(range(B)),
                                          trace=False)
    out = np.empty((B, N, HID), dtype=np.float32)
    for c in range(B):
        outT = res.results[c]["outT"].astype(np.float32)
        w = res.results[c]["wsum"].astype(np.float32)
        w = w.reshape(NCH, NG * P, GS // 2, ICH)
        rowsum = w.sum(axis=(1, 2)).reshape(1, N)
        rowsum = np.where(rowsum == 0.0, 1.0, rowsum)
        out[c] = (outT / rowsum).T
    return out


# revision 26
# speedup vs baseline: 1.1608x; 1.1317x over previous
"""Trainium2 Bass kernel for nn_AttModel (masked attention GNN message passing).

Contract: kernel(**inputs) takes the FULL unsharded inputs (x [8,2048,128],
mask [8,2048,2048], Wv/Wk/Wq [128,128], bv/bk/bq [128]) and returns the full
output [8, 2048, 128] float32.

Strategy: data-parallel over batch B=8 across the 8 NeuronCores; weights
replicated. Per core, fully transposed dataflow (scores computed as S^T):

  qT/kT/vT = relu(W xT + b) as [h, n] bf16  (bf16 projections, f32 PSUM)
  v_all    = vT block-transposed to [n, h] via PE transposes
  per i-chunk (1024) x j-stripe-group (4 stripes of 128):
    sT   = kT_j^T @ qT_chunk          (PE, bf16, PSUM f32)
    e    = exp(sT)                    (ACT, PSUM -> SBUF bf16)
    p    = e "masked to ~0"           (applied DURING the mask DMA via
                                       SWDGE accum_op=add on the int16
                                       BIT PATTERN of e -- mask never
                                       touches a compute engine)
    outT += v_j^T @ p                 (PE, PSUM accumulation over stripes)
    tree: w_g = (p0+p1)+(p2+p3)       (DVE pair-sum)
    rowsum += 1^T @ w_g               (PE, M=1 matmul on tree output)
  Host: out_b = (outT / rowsum)^T

The group back-end (out matmuls + tree + rowsum) is software-pipelined one
group behind the front-end (scores + exp + mask DMA) so the PE never
head-of-line blocks on the mask DMA completion and stays HAM-warm.

Masking trick: q,k >= 0 post-relu so s >= 0 and e = exp(s) >= 1.0, i.e. the
bf16 bits of e are >= 0x3F80. The mask ships as int16 {0, -0x3F80}; the SWDGE
DMA integer-adds it onto the bitcast e tile, so masked entries become
bits-0x3F80 = e * 2^-127 (~1e-30, vanishes in f32 accumulation against
unmasked terms >= 1) while unmasked entries are untouched (+0 exact).
"""

from contextlib import ExitStack

import numpy as np
import ml_dtypes

import concourse.bass as bass
import concourse.bacc as bacc
import concourse.tile as tile
from concourse import mybir
from concourse import bass_utils

B = 8
P = 128
N = 2048
HID = 128
DIN = 128
ICH = 1024       # i-chunk width
NCH = N // ICH   # 2 i-chunks
NG = 4           # stripe groups per chunk
GS = 4           # stripes per group

f32 = mybir.dt.float32
bf = mybir.dt.bfloat16
i16 = mybir.dt.int16
AF = mybir.ActivationFunctionType
ALU = mybir.AluOpType

MASK_SUB = -0x3F80  # subtract from bf16 bits of e where mask==0

_NC_CACHE = {}


def _attention_tile_kernel(ctx, tc, outT, wsum, xn, maskh, cf32, cbf):
    nc = tc.nc

    consts = ctx.enter_context(tc.tile_pool(name="consts", bufs=1))
    big = ctx.enter_context(tc.tile_pool(name="big", bufs=1))

    # x arrives pre-transposed [d, n] from the host; plain DMAs (the
    # DMA-transpose path serializes against all other DMAs). First half
    # gates the first projections, so it is issued first and split.
    xT_sb = big.tile([P, N], bf)
    nc.sync.dma_start(out=xT_sb[:, 0:ICH], in_=xn[:, 0:ICH])
    blob_b = consts.tile([P, 512], bf)
    nc.scalar.dma_start(out=blob_b, in_=cbf)
    blob_f = consts.tile([P, 4], f32)
    nc.scalar.dma_start(out=blob_f, in_=cf32)
    nc.sync.dma_start(out=xT_sb[:, ICH:N], in_=xn[:, ICH:N])

    biases = {"q": blob_f[:, 0:1], "k": blob_f[:, 1:2], "v": blob_f[:, 2:3]}
    wTs = {"q": blob_b[:, 0:P], "k": blob_b[:, P:2 * P], "v": blob_b[:, 2 * P:3 * P]}
    idb = blob_b[:, 3 * P:4 * P]

    mask_pool = ctx.enter_context(tc.tile_pool(name="maskp", bufs=1))
    # Prefetch the per-group q3 mask slices on the plain (fast) DMA path;
    # they have no dependencies so they load during the projection preamble.
    mask_dve = {}
    for cc in range(NCH):
        for gg in range(NG):
            qs = range(GS) if (cc * NG + gg) >= NCH * NG - 2 else (3,)
            for qq in qs:
                mt = mask_pool.tile([P, ICH], i16, tag=f"m{cc}{gg}{qq}",
                                    name=f"mask_{cc}_{gg}_{qq}")
                j0 = (gg * GS + qq) * P
                nc.sync.dma_start(
                    out=mt, in_=maskh[j0:j0 + P, cc * ICH:(cc + 1) * ICH])
                mask_dve[(cc, gg, qq)] = mt

    qT = big.tile([P, N], bf)
    kT = big.tile([P, N], bf)
    vT = big.tile([P, N], bf)
    v_all = big.tile([P, N], bf)    # [n, h] in 128-blocks: v_all[:, jt*128+h]

    s_psum = ctx.enter_context(tc.tile_pool(name="spsum", bufs=2, space="PSUM"))
    o_psum = ctx.enter_context(tc.tile_pool(name="opsum", bufs=1, space="PSUM"))
    p_psum = ctx.enter_context(tc.tile_pool(name="ppsum", bufs=1, space="PSUM"))
    e_pool = ctx.enter_context(tc.tile_pool(name="ep", bufs=5))
    tree_pool = ctx.enter_context(tc.tile_pool(name="treep", bufs=2))
    out_sb_pool = ctx.enter_context(tc.tile_pool(name="outsbp", bufs=2))

    def proj(nm, c, dest, act_engine):
        """dest[:, c*1024:(c+1)*1024] = relu(W^T x + b) for n-chunk c."""
        ps = p_psum.tile([P, ICH], f32, tag="p", name=f"proj_{nm}{c}")
        for h in range(2):
            nc.tensor.matmul(ps[:, h * 512:(h + 1) * 512], lhsT=wTs[nm],
                             rhs=xT_sb[:, c * ICH + h * 512:c * ICH + (h + 1) * 512],
                             start=True, stop=True)
        dslice = dest[:, c * ICH:(c + 1) * ICH]
        if act_engine:
            nc.scalar.activation(out=dslice, in_=ps, func=AF.Relu,
                                 bias=biases[nm], scale=1.0)
        else:
            nc.vector.tensor_scalar(out=dslice, in0=ps, scalar1=biases[nm],
                                    scalar2=0.0, op0=ALU.add, op1=ALU.max)

    def transp_half(half):
        """v_all[:, half*1024:(half+1)*1024] = blockwise vT^T (8 blocks)."""
        tp = p_psum.tile([P, ICH], bf, tag="p", name=f"vtp{half}")
        for b8 in range(8):
            jt = half * 8 + b8
            nc.tensor.transpose(tp[:, b8 * P:(b8 + 1) * P],
                                vT[:, jt * P:(jt + 1) * P], idb)
        nc.vector.tensor_copy(out=v_all[:, half * ICH:(half + 1) * ICH], in_=tp)

    o_tiles = {}
    e_tiles = {}

    def front(c, g):
        """Scores + exp for 4 stripes, then the masking DMA."""
        i0 = c * ICH
        e_big = e_pool.tile([P, GS * ICH], bf, tag="e", name=f"e_{c}_{g}")
        e_tiles[(c, g)] = e_big
        for q in range(GS):
            jt = g * GS + q
            s_ps = s_psum.tile([P, ICH], f32, tag="s", name=f"s_{c}_{jt}")
            for h in range(2):
                nc.tensor.matmul(
                    s_ps[:, h * 512:(h + 1) * 512],
                    lhsT=kT[:, jt * P:(jt + 1) * P],
                    rhs=qT[:, i0 + h * 512:i0 + (h + 1) * 512],
                    start=True, stop=True)
            nc.scalar.activation(out=e_big[:, q * ICH:(q + 1) * ICH],
                                 in_=s_ps, func=AF.Exp)
        # Apply the mask: stripes q0..q2 int16-add {0,-0x3F80} onto the
        # bf16 bit patterns of e DURING the DMA (CCE accumulate path, ~200
        # GB/s aggregate); stripe q3 via a DVE int16-add against the
        # prefetched plain-path mask tile. The accum source runs must be
        # strided (adjacent runs aggregate past the CCE element limit).
        # The FINAL group masks all four stripes on the DVE so the tail
        # never waits on the slow accumulate-DMA.
        last = (c * NG + g) >= NCH * NG - 2
        if not last:
            m_ap = maskh[g * GS * P:(g * GS + 3) * P,
                         i0:i0 + ICH].rearrange("(s p) i -> p s i", p=P)
            nc.gpsimd.dma_start(out=e_big[:, 0:3 * ICH].bitcast(i16),
                                in_=m_ap, accum_op=ALU.add)
        for qq in (range(GS) if last else (3,)):
            eq = e_big[:, qq * ICH:(qq + 1) * ICH].bitcast(i16)
            nc.vector.tensor_tensor(out=eq, in0=eq,
                                    in1=mask_dve.pop((c, g, qq)), op=ALU.add)

    def back(c, g):
        """Out-matmul accumulation + rowsum tree for a completed group."""
        e_big = e_tiles.pop((c, g))
        if g == 0:
            o_tiles[c] = o_psum.tile([P, ICH], f32, tag="o", name=f"o_{c}")
        o_ps = o_tiles[c]
        for q in range(GS):
            jt = g * GS + q
            for h in range(2):
                nc.tensor.matmul(
                    o_ps[:, h * 512:(h + 1) * 512],
                    lhsT=v_all[:, jt * P:(jt + 1) * P],
                    rhs=e_big[:, q * ICH + h * 512:q * ICH + (h + 1) * 512],
                    start=(jt == 0), stop=(jt == 15))
        t_g = tree_pool.tile([P, 2 * ICH], bf, tag="t", name=f"t_{c}_{g}")
        nc.vector.tensor_tensor(out=t_g, in0=e_big[:, 0:2 * ICH],
                                in1=e_big[:, 2 * ICH:4 * ICH], op=ALU.add)
        w_g = tree_pool.tile([P, ICH], bf, tag="w", name=f"w_{c}_{g}")
        nc.vector.tensor_tensor(out=w_g, in0=t_g[:, 0:ICH],
                                in1=t_g[:, ICH:2 * ICH], op=ALU.add)
        # the final 128-way partition reduction of w_g happens on the host
        widx = (c * NG + g) * P
        nc.sync.dma_start(out=wsum[widx:widx + P, :], in_=w_g)

    def flush(c):
        i0 = c * ICH
        o_ps = o_tiles.pop(c)
        out_sb = out_sb_pool.tile([P, ICH], bf, tag="osb", name=f"osb_{c}")
        if c == NCH - 1:
            # ACT is idle once the final exp retires
            nc.scalar.copy(out=out_sb, in_=o_ps)
        else:
            nc.vector.tensor_copy(out=out_sb, in_=o_ps)
        nc.sync.dma_start(out=outT[:, i0:i0 + ICH], in_=out_sb)

    # Critical path into the main loop: q/k chunk-0 projections (ACT relu).
    proj("q", 0, qT, True)
    proj("k", 0, kT, True)

    # Off-critical-path setup, interleaved into the early groups (the PE has
    # spare cycles there since the back-end lags by one group).
    extras = {
        (0, 0): [lambda: proj("v", 0, vT, False), lambda: transp_half(0)],
        (0, 1): [lambda: proj("q", 1, qT, False),
                 lambda: proj("k", 1, kT, False)],
        (0, 2): [lambda: proj("v", 1, vT, False), lambda: transp_half(1)],
    }

    # Explicit scheduling windows: tile_set_cur_wait pins each pipeline
    # stage into its own window in the scheduler's simulated timeline, so
    # group g's scores/exp always precede the (lagged) back-end of group
    # g-LAG in every engine queue -- the scheduler's DMA cost model
    # underestimates the accumulate path and would otherwise reorder.
    LAG = 3
    seq = [(c, g) for c in range(NCH) for g in range(NG)]
    for idx, cg in enumerate(seq):
        tc.tile_set_cur_wait(1.0 * (1 + idx))
        front(*cg)
        for fn in extras.get(cg, ()):
            fn()
        if idx >= LAG:
            done = seq[idx - LAG]
            back(*done)
            if done[1] == NG - 1:
                flush(done[0])
    for j, done in enumerate(seq[-LAG:]):
        tc.tile_set_cur_wait(1.0 * (1 + len(seq) + j))
        back(*done)
        if done[1] == NG - 1:
            flush(done[0])


def _build_nc():
    if "nc" in _NC_CACHE:
        return _NC_CACHE["nc"]
    nc = bacc.Bacc("TRN2", target_bir_lowering=False, debug=False, num_devices=B)
    xn = nc.dram_tensor("xn", [DIN, N], bf, kind="ExternalInput").ap()
    maskh = nc.dram_tensor("maskh", [N, N], i16, kind="ExternalInput").ap()
    cf32 = nc.dram_tensor("cf32", [P, 4], f32, kind="ExternalInput").ap()
    cbf = nc.dram_tensor("cbf", [P, 512], bf, kind="ExternalInput").ap()
    outT = nc.dram_tensor("outT", [HID, N], bf, kind="ExternalOutput").ap()
    wsum = nc.dram_tensor("wsum", [NCH * NG * P, ICH], bf,
                          kind="ExternalOutput").ap()

    with tile.TileContext(nc) as tc:
        with ExitStack() as ctx:
            _attention_tile_kernel(ctx, tc, outT, wsum, xn, maskh, cf32, cbf)
    nc.compile()
    _NC_CACHE["nc"] = nc
    return nc


def build_nc():
    return _build_nc()


def make_in_maps(x, mask, Wv, bv, Wk, bk, Wq, bq):
    x = np.asarray(x, dtype=np.float32)
    mask = np.asarray(mask, dtype=np.float32)
    Wv = np.asarray(Wv, dtype=np.float32)
    bv = np.asarray(bv, dtype=np.float32)
    Wk = np.asarray(Wk, dtype=np.float32)
    bk = np.asarray(bk, dtype=np.float32)
    Wq = np.asarray(Wq, dtype=np.float32)
    bq = np.asarray(bq, dtype=np.float32)

    cf32 = np.zeros((P, 4), np.float32)
    cf32[:, 0], cf32[:, 1], cf32[:, 2] = bq, bk, bv
    cbf = np.concatenate(
        [Wq.T, Wk.T, Wv.T, np.eye(P, dtype=np.float32)],
        axis=1).astype(ml_dtypes.bfloat16)
    cbf = np.ascontiguousarray(cbf)

    in_maps = []
    for c in range(B):
        maskadd = np.where(mask[c].T >= 0.5, 0, MASK_SUB).astype(np.int16)
        in_maps.append({
            "xn": np.ascontiguousarray(x[c].T).astype(ml_dtypes.bfloat16),
            "maskh": np.ascontiguousarray(maskadd),
            "cf32": cf32, "cbf": cbf,
        })
    return in_maps


def kernel(x, mask, Wv, bv, Wk, bk, Wq, bq):
    nc = _build_nc()
    in_maps = make_in_maps(x, mask, Wv, bv, Wk, bk, Wq, bq)
    res = bass_utils.run_bass_kernel_spmd(nc, in_maps, core_ids=list(range(B)),
                                          trace=False)
    out = np.empty((B, N, HID), dtype=np.float32)
    for c in range(B):
        outT = res.results[c]["outT"].astype(np.float32)
        w = res.results[c]["wsum"].astype(np.float32)
        w = w.reshape(NCH, NG * P, ICH)
        rowsum = w.sum(axis=1).reshape(1, N)
        rowsum = np.where(rowsum == 0.0, 1.0, rowsum)
        out[c] = (outT / rowsum).T
    return out


# revision 27
# speedup vs baseline: 1.2400x; 1.0682x over previous
"""Trainium2 Bass kernel for nn_AttModel (masked attention GNN message passing).

Contract: kernel(**inputs) takes the FULL unsharded inputs (x [8,2048,128],
mask [8,2048,2048], Wv/Wk/Wq [128,128], bv/bk/bq [128]) and returns the full
output [8, 2048, 128] float32.

Strategy: data-parallel over batch B=8 across the 8 NeuronCores; weights
replicated. Per core, fully transposed dataflow (scores computed as S^T):

  qT/kT/vT = relu(W xT + b) as [h, n] bf16  (bf16 projections, f32 PSUM)
  v_all    = vT block-transposed to [n, h] via PE transposes
  per i-chunk (1024) x j-stripe-group (4 stripes of 128):
    sT   = kT_j^T @ qT_chunk          (PE, bf16, PSUM f32)
    e    = exp(sT)                    (ACT, PSUM -> SBUF bf16)
    p    = e "masked to ~0"           (applied DURING the mask DMA via
                                       SWDGE accum_op=add on the int16
                                       BIT PATTERN of e -- mask never
                                       touches a compute engine)
    outT += v_j^T @ p                 (PE, PSUM accumulation over stripes)
    tree: w_g = (p0+p1)+(p2+p3)       (DVE pair-sum)
    rowsum += 1^T @ w_g               (PE, M=1 matmul on tree output)
  Host: out_b = (outT / rowsum)^T

The group back-end (out matmuls + tree + rowsum) is software-pipelined one
group behind the front-end (scores + exp + mask DMA) so the PE never
head-of-line blocks on the mask DMA completion and stays HAM-warm.

Masking trick: q,k >= 0 post-relu so s >= 0 and e = exp(s) >= 1.0, i.e. the
bf16 bits of e are >= 0x3F80. The mask ships as int16 {0, -0x3F80}; the SWDGE
DMA integer-adds it onto the bitcast e tile, so masked entries become
bits-0x3F80 = e * 2^-127 (~1e-30, vanishes in f32 accumulation against
unmasked terms >= 1) while unmasked entries are untouched (+0 exact).
"""

from contextlib import ExitStack

import numpy as np
import ml_dtypes

import concourse.bass as bass
import concourse.bacc as bacc
import concourse.tile as tile
from concourse import mybir
from concourse import bass_utils

B = 8
P = 128
N = 2048
HID = 128
DIN = 128
ICH = 1024       # i-chunk width
NCH = N // ICH   # 2 i-chunks
NG = 4           # stripe groups per chunk
GS = 4           # stripes per group

f32 = mybir.dt.float32
bf = mybir.dt.bfloat16
i16 = mybir.dt.int16
AF = mybir.ActivationFunctionType
ALU = mybir.AluOpType

MASK_SUB = -0x3F80  # subtract from bf16 bits of e where mask==0

_NC_CACHE = {}


def _attention_tile_kernel(ctx, tc, outT, wsum, xn, maskh, cf32, cbf):
    nc = tc.nc

    consts = ctx.enter_context(tc.tile_pool(name="consts", bufs=1))
    big = ctx.enter_context(tc.tile_pool(name="big", bufs=1))

    # x arrives pre-transposed [d, n] from the host; plain DMAs (the
    # DMA-transpose path serializes against all other DMAs). First half
    # gates the first projections, so it is issued first and split.
    xT_sb = big.tile([P, N], bf)
    nc.sync.dma_start(out=xT_sb[:, 0:ICH], in_=xn[:, 0:ICH])
    blob_b = consts.tile([P, 512], bf)
    nc.scalar.dma_start(out=blob_b, in_=cbf)
    blob_f = consts.tile([P, 4], f32)
    nc.scalar.dma_start(out=blob_f, in_=cf32)
    nc.sync.dma_start(out=xT_sb[:, ICH:N], in_=xn[:, ICH:N])

    biases = {"q": blob_f[:, 0:1], "k": blob_f[:, 1:2], "v": blob_f[:, 2:3]}
    wTs = {"q": blob_b[:, 0:P], "k": blob_b[:, P:2 * P], "v": blob_b[:, 2 * P:3 * P]}
    idb = blob_b[:, 3 * P:4 * P]

    mask_pool = ctx.enter_context(tc.tile_pool(name="maskp", bufs=1))
    # Prefetch the per-group q3 mask slices on the plain (fast) DMA path;
    # they have no dependencies so they load during the projection preamble.
    mask_dve = {}
    for cc in range(NCH):
        for gg in range(NG):
            qs = range(GS) if (cc * NG + gg) >= NCH * NG - 2 else (3,)
            for qq in qs:
                mt = mask_pool.tile([P, ICH], i16, tag=f"m{cc}{gg}{qq}",
                                    name=f"mask_{cc}_{gg}_{qq}")
                j0 = (gg * GS + qq) * P
                nc.sync.dma_start(
                    out=mt, in_=maskh[j0:j0 + P, cc * ICH:(cc + 1) * ICH])
                mask_dve[(cc, gg, qq)] = mt

    qT = big.tile([P, N], bf)
    kT = big.tile([P, N], bf)
    vT = big.tile([P, N], bf)
    v_all = big.tile([P, N], bf)    # [n, h] in 128-blocks: v_all[:, jt*128+h]

    s_psum = ctx.enter_context(tc.tile_pool(name="spsum", bufs=2, space="PSUM"))
    o_psum = ctx.enter_context(tc.tile_pool(name="opsum", bufs=1, space="PSUM"))
    p_psum = ctx.enter_context(tc.tile_pool(name="ppsum", bufs=1, space="PSUM"))
    e_pool = ctx.enter_context(tc.tile_pool(name="ep", bufs=5))
    tree_pool = ctx.enter_context(tc.tile_pool(name="treep", bufs=2))
    out_sb_pool = ctx.enter_context(tc.tile_pool(name="outsbp", bufs=2))

    def proj(nm, c, dest, act_engine, pool_tag="p"):
        """dest[:, c*1024:(c+1)*1024] = relu(W^T x + b) for n-chunk c."""
        pool = p_psum if pool_tag == "p" else s_psum
        ps = pool.tile([P, ICH], f32, tag=pool_tag, name=f"proj_{nm}{c}")
        for h in range(2):
            nc.tensor.matmul(ps[:, h * 512:(h + 1) * 512], lhsT=wTs[nm],
                             rhs=xT_sb[:, c * ICH + h * 512:c * ICH + (h + 1) * 512],
                             start=True, stop=True)
        dslice = dest[:, c * ICH:(c + 1) * ICH]
        if act_engine:
            nc.scalar.activation(out=dslice, in_=ps, func=AF.Relu,
                                 bias=biases[nm], scale=1.0)
        else:
            nc.vector.tensor_scalar(out=dslice, in0=ps, scalar1=biases[nm],
                                    scalar2=0.0, op0=ALU.add, op1=ALU.max)

    def transp_half(half):
        """v_all[:, half*1024:(half+1)*1024] = blockwise vT^T (8 blocks)."""
        tp = p_psum.tile([P, ICH], bf, tag="p", name=f"vtp{half}")
        for b8 in range(8):
            jt = half * 8 + b8
            nc.tensor.transpose(tp[:, b8 * P:(b8 + 1) * P],
                                vT[:, jt * P:(jt + 1) * P], idb)
        nc.vector.tensor_copy(out=v_all[:, half * ICH:(half + 1) * ICH], in_=tp)

    o_tiles = {}
    e_tiles = {}

    def front(c, g):
        """Scores + exp for 4 stripes, then the masking DMA."""
        i0 = c * ICH
        e_big = e_pool.tile([P, GS * ICH], bf, tag="e", name=f"e_{c}_{g}")
        e_tiles[(c, g)] = e_big
        for q in range(GS):
            jt = g * GS + q
            # chunk 1 borrows the (setup-dead) projection pool as a third
            # score buffer, deepening the rotation past the bufs=2 limit
            if c == 1 and jt % 3 == 2:
                s_ps = p_psum.tile([P, ICH], f32, tag="p", name=f"s_{c}_{jt}")
            else:
                s_ps = s_psum.tile([P, ICH], f32, tag="s", name=f"s_{c}_{jt}")
            for h in range(2):
                nc.tensor.matmul(
                    s_ps[:, h * 512:(h + 1) * 512],
                    lhsT=kT[:, jt * P:(jt + 1) * P],
                    rhs=qT[:, i0 + h * 512:i0 + (h + 1) * 512],
                    start=True, stop=True)
            nc.scalar.activation(out=e_big[:, q * ICH:(q + 1) * ICH],
                                 in_=s_ps, func=AF.Exp)
        # Apply the mask: stripes q0..q2 int16-add {0,-0x3F80} onto the
        # bf16 bit patterns of e DURING the DMA (CCE accumulate path, ~200
        # GB/s aggregate); stripe q3 via a DVE int16-add against the
        # prefetched plain-path mask tile. The accum source runs must be
        # strided (adjacent runs aggregate past the CCE element limit).
        # The FINAL group masks all four stripes on the DVE so the tail
        # never waits on the slow accumulate-DMA.
        last = (c * NG + g) >= NCH * NG - 2
        if not last:
            m_ap = maskh[g * GS * P:(g * GS + 3) * P,
                         i0:i0 + ICH].rearrange("(s p) i -> p s i", p=P)
            nc.gpsimd.dma_start(out=e_big[:, 0:3 * ICH].bitcast(i16),
                                in_=m_ap, accum_op=ALU.add)
        for qq in (range(GS) if last else (3,)):
            eq = e_big[:, qq * ICH:(qq + 1) * ICH].bitcast(i16)
            nc.vector.tensor_tensor(out=eq, in0=eq,
                                    in1=mask_dve.pop((c, g, qq)), op=ALU.add)

    def back(c, g):
        """Out-matmul accumulation + rowsum tree for a completed group."""
        e_big = e_tiles.pop((c, g))
        if g == 0:
            o_tiles[c] = o_psum.tile([P, ICH], f32, tag="o", name=f"o_{c}")
        o_ps = o_tiles[c]
        for q in range(GS):
            jt = g * GS + q
            for h in range(2):
                nc.tensor.matmul(
                    o_ps[:, h * 512:(h + 1) * 512],
                    lhsT=v_all[:, jt * P:(jt + 1) * P],
                    rhs=e_big[:, q * ICH + h * 512:q * ICH + (h + 1) * 512],
                    start=(jt == 0), stop=(jt == 15))
        t_g = tree_pool.tile([P, 2 * ICH], bf, tag="t", name=f"t_{c}_{g}")
        nc.vector.tensor_tensor(out=t_g, in0=e_big[:, 0:2 * ICH],
                                in1=e_big[:, 2 * ICH:4 * ICH], op=ALU.add)
        w_g = tree_pool.tile([P, ICH], bf, tag="w", name=f"w_{c}_{g}")
        nc.vector.tensor_tensor(out=w_g, in0=t_g[:, 0:ICH],
                                in1=t_g[:, ICH:2 * ICH], op=ALU.add)
        # the final 128-way partition reduction of w_g happens on the host
        widx = (c * NG + g) * P
        nc.sync.dma_start(out=wsum[widx:widx + P, :], in_=w_g)

    def flush(c):
        i0 = c * ICH
        o_ps = o_tiles.pop(c)
        out_sb = out_sb_pool.tile([P, ICH], bf, tag="osb", name=f"osb_{c}")
        if c == NCH - 1:
            # ACT is idle once the final exp retires
            nc.scalar.copy(out=out_sb, in_=o_ps)
        else:
            nc.vector.tensor_copy(out=out_sb, in_=o_ps)
        nc.sync.dma_start(out=outT[:, i0:i0 + ICH], in_=out_sb)

    # Critical path into the main loop: q relu on ACT, k relu on the DVE
    # through the s-pool -- the two run in parallel.
    proj("q", 0, qT, True)
    proj("k", 0, kT, False, pool_tag="s")

    # Off-critical-path setup, interleaved into the early groups (the PE has
    # spare cycles there since the back-end lags by one group).
    extras = {
        (0, 0): [lambda: proj("v", 0, vT, False), lambda: transp_half(0)],
        (0, 1): [lambda: proj("q", 1, qT, False),
                 lambda: proj("k", 1, kT, False)],
        (0, 2): [lambda: proj("v", 1, vT, False), lambda: transp_half(1)],
    }

    # Explicit scheduling windows: tile_set_cur_wait pins each pipeline
    # stage into its own window in the scheduler's simulated timeline, so
    # group g's scores/exp always precede the (lagged) back-end of group
    # g-LAG in every engine queue -- the scheduler's DMA cost model
    # underestimates the accumulate path and would otherwise reorder.
    LAG = 3
    seq = [(c, g) for c in range(NCH) for g in range(NG)]
    for idx, cg in enumerate(seq):
        tc.tile_set_cur_wait(1.0 * (1 + idx))
        front(*cg)
        for fn in extras.get(cg, ()):
            fn()
        if idx >= LAG:
            done = seq[idx - LAG]
            back(*done)
            if done[1] == NG - 1:
                flush(done[0])
    for j, done in enumerate(seq[-LAG:]):
        tc.tile_set_cur_wait(1.0 * (1 + len(seq) + j))
        back(*done)
        if done[1] == NG - 1:
            flush(done[0])


def _build_nc():
    if "nc" in _NC_CACHE:
        return _NC_CACHE["nc"]
    nc = bacc.Bacc("TRN2", target_bir_lowering=False, debug=False, num_devices=B)
    xn = nc.dram_tensor("xn", [DIN, N], bf, kind="ExternalInput").ap()
    maskh = nc.dram_tensor("maskh", [N, N], i16, kind="ExternalInput").ap()
    cf32 = nc.dram_tensor("cf32", [P, 4], f32, kind="ExternalInput").ap()
    cbf = nc.dram_tensor("cbf", [P, 512], bf, kind="ExternalInput").ap()
    outT = nc.dram_tensor("outT", [HID, N], bf, kind="ExternalOutput").ap()
    wsum = nc.dram_tensor("wsum", [NCH * NG * P, ICH], bf,
                          kind="ExternalOutput").ap()

    with tile.TileContext(nc) as tc:
        with ExitStack() as ctx:
            _attention_tile_kernel(ctx, tc, outT, wsum, xn, maskh, cf32, cbf)
    nc.compile()
    _NC_CACHE["nc"] = nc
    return nc


def build_nc():
    return _build_nc()


def make_in_maps(x, mask, Wv, bv, Wk, bk, Wq, bq):
    x = np.asarray(x, dtype=np.float32)
    mask = np.asarray(mask, dtype=np.float32)
    Wv = np.asarray(Wv, dtype=np.float32)
    bv = np.asarray(bv, dtype=np.float32)
    Wk = np.asarray(Wk, dtype=np.float32)
    bk = np.asarray(bk, dtype=np.float32)
    Wq = np.asarray(Wq, dtype=np.float32)
    bq = np.asarray(bq, dtype=np.float32)

    cf32 = np.zeros((P, 4), np.float32)
    cf32[:, 0], cf32[:, 1], cf32[:, 2] = bq, bk, bv
    cbf = np.concatenate(
        [Wq.T, Wk.T, Wv.T, np.eye(P, dtype=np.float32)],
        axis=1).astype(ml_dtypes.bfloat16)
    cbf = np.ascontiguousarray(cbf)

    in_maps = []
    for c in range(B):
        maskadd = np.where(mask[c].T >= 0.5, 0, MASK_SUB).astype(np.int16)
        in_maps.append({
            "xn": np.ascontiguousarray(x[c].T).astype(ml_dtypes.bfloat16),
            "maskh": np.ascontiguousarray(maskadd),
            "cf32": cf32, "cbf": cbf,
        })
    return in_maps


def kernel(x, mask, Wv, bv, Wk, bk, Wq, bq):
    nc = _build_nc()
    in_maps = make_in_maps(x, mask, Wv, bv, Wk, bk, Wq, bq)
    res = bass_utils.run_bass_kernel_spmd(nc, in_maps, core_ids=list(range(B)),
                                          trace=False)
    out = np.empty((B, N, HID), dtype=np.float32)
    for c in range(B):
        outT = res.results[c]["outT"].astype(np.float32)
        w = res.results[c]["wsum"].astype(np.float32)
        w = w.reshape(NCH, NG * P, ICH)
        rowsum = w.sum(axis=1).reshape(1, N)
        rowsum = np.where(rowsum == 0.0, 1.0, rowsum)
        out[c] = (outT / rowsum).T
    return out
